# revision 8
# baseline (speedup 1.0000x reference)
"""GCN message-passing kernel for 8 Trainium2 NeuronCores.

Layout/design:
- Nodes sharded 6250/core. Each core computes segment sums (spmm) for its own
  src rows; gathers source-side features from a full replicated table in DRAM
  that is refreshed by AllGather each half-layer.
- Feature tables live in DRAM in a per-core tiled layout: within core r's
  block, row p*NBLK + t holds node r*S + t*128 + p. Gather indices are
  precomputed host-side against this layout.
- spmm: edges sorted by (block of src, table-half of dst); dma_gather fetches
  64-float rows (256B) per edge in batches; a per-128-edge-chunk selection
  matrix (built on device with one is_equal vs an iota constant) is matmul'd
  on the PE against the w-scaled gathered rows, accumulating each block's
  [128, 64] segment sums in PSUM.
- Per-(block,half) chunk counts are padded to the max over cores so all 8
  cores run one identical instruction stream (one NEFF, SPMD).
"""

import numpy as np

P = 128
EMB = 64
F_IN = 32
BATCH = 8  # chunks (of 128 edges) per dma_gather call (1024 idxs; ring-capacity-safe)
SCALE = 0.4251202479144762
N_LAYERS = 4

TRACE = False
LAST_RESULT = None


# ---------------------------------------------------------------- host prep


def _prep_edges(edge_index, edge_weight, n, ncores):
    """Sort/pad edges into per-core uniform gather streams.

    Returns dict with per-core streams and the shared chunk schedule.
    """
    s = n // ncores          # rows per core
    nblk = (s + P - 1) // P  # 128-row blocks per core
    tabr = nblk * P          # table rows per core (padded)
    half_rows = tabr * ncores // 2
    assert half_rows <= 32768, half_rows

    src = np.asarray(edge_index[:, 0], dtype=np.int64)
    dst = np.asarray(edge_index[:, 1], dtype=np.int64)
    w = np.asarray(edge_weight, dtype=np.float32)

    # table row for node v (interleaved node->core: core = v % ncores)
    r = dst % ncores
    sloc = dst // ncores
    trow = r * tabr + (sloc % P) * nblk + sloc // P
    half = (trow >= half_rows).astype(np.int64)
    idx16 = (trow - half * half_rows).astype(np.int16)

    core = src % ncores
    src_l = src // ncores
    blk = src_l // P
    srcf = (src_l % P).astype(np.float32)

    # bucket counts: cnt[core, half, blk]
    cnt = np.zeros((ncores, 2, nblk), dtype=np.int64)
    np.add.at(cnt, (core, half, blk), 1)
    nch = np.maximum(
        (cnt + P - 1) // P, 0
    ).max(axis=0)  # [2, nblk] chunks per (half, blk), uniform over cores
    c_h = nch.sum(axis=1)  # [2] chunks per half
    l_h = c_h * P

    # chunk offsets per (half, blk)
    off = np.zeros((2, nblk), dtype=np.int64)
    off[:, 1:] = np.cumsum(nch, axis=1)[:, :-1]

    # per-core streams
    streams = []
    for c in range(ncores):
        m = core == c
        o = np.lexsort((dst[m], blk[m], half[m]))
        eh, eb = half[m][o], blk[m][o]
        ei, ew_, es = idx16[m][o], w[m][o], srcf[m][o]
        idx_s = [np.zeros(l_h[h], np.int16) for h in range(2)]
        w_s = [np.zeros(l_h[h], np.float32) for h in range(2)]
        sf_s = [np.zeros(l_h[h], np.float32) for h in range(2)]
        pos = 0
        for h in range(2):
            for b in range(nblk):
                k = int(cnt[c, h, b])
                if k:
                    lo = off[h, b] * P
                    idx_s[h][lo : lo + k] = ei[pos : pos + k]
                    w_s[h][lo : lo + k] = ew_[pos : pos + k]
                    sf_s[h][lo : lo + k] = es[pos : pos + k]
                    pos += k
        assert pos == int(m.sum())
        streams.append((idx_s, w_s, sf_s))

    # shared chunk schedule: per block, list of (half, chunk_index_in_half)
    block_chunks = []
    for b in range(nblk):
        lst = []
        for h in range(2):
            for j in range(int(nch[h, b])):
                lst.append((h, int(off[h, b] + j)))
        block_chunks.append(lst)

    return {
        "s": s,
        "nblk": nblk,
        "tabr": tabr,
        "half_rows": half_rows,
        "c_h": [int(c_h[0]), int(c_h[1])],
        "l_h": [int(l_h[0]), int(l_h[1])],
        "block_chunks": block_chunks,
        "streams": streams,
    }


def _tile_rows(a, s, nblk, width):
    """[s, width] row-major -> [128, nblk*width] tiled (p, t) layout,
    zero-padded."""
    out = np.zeros((P, nblk * width), a.dtype)
    rows = np.arange(s)
    p, t = rows % P, rows // P
    for k in range(width):
        out[p, t * width + k] = a[rows, k]
    return out


def _idx_layout(stream):
    """int16 stream -> [128, len/16] replicated-by-16 layout."""
    return np.tile(stream.reshape(-1, 16).T, (8, 1)).copy()


def _chunk_layout(stream):
    """f32 stream -> [128, n_chunks]: element (chunk*128+p) -> [p, chunk]."""
    return np.ascontiguousarray(stream.reshape(-1, P).T)


# ---------------------------------------------------------------- bass build


def _build(meta, scalars, ncores):
    import concourse.bacc as bacc
    import concourse.mybir as mybir
    import concourse.tile as tile

    fp32 = mybir.dt.float32
    bf16 = mybir.dt.bfloat16
    i16 = mybir.dt.int16

    s, nblk, tabr = meta["s"], meta["nblk"], meta["tabr"]
    half_rows = meta["half_rows"]
    c_h, l_h = meta["c_h"], meta["l_h"]
    block_chunks = meta["block_chunks"]
    n_e = scalars["n_edges"]
    t_ctov, t_vtoc = scalars["t_ctov"], scalars["t_vtoc"]

    nc = bacc.Bacc("TRN2", target_bir_lowering=False, debug=False,
                   num_devices=ncores)

    # ---- kernel IO
    ein = {}
    ein["idx0"] = nc.dram_tensor("idx0", [P, l_h[0] // 16], i16, kind="ExternalInput")
    ein["idx1"] = nc.dram_tensor("idx1", [P, l_h[1] // 16], i16, kind="ExternalInput")
    ein["w0"] = nc.dram_tensor("w0", [P, c_h[0]], fp32, kind="ExternalInput")
    ein["w1"] = nc.dram_tensor("w1", [P, c_h[1]], fp32, kind="ExternalInput")
    ein["sf0"] = nc.dram_tensor("sf0", [P, c_h[0]], fp32, kind="ExternalInput")
    ein["sf1"] = nc.dram_tensor("sf1", [P, c_h[1]], fp32, kind="ExternalInput")
    ein["cf"] = nc.dram_tensor("cf", [P, nblk * F_IN], fp32, kind="ExternalInput")
    ein["vf"] = nc.dram_tensor("vf", [P, nblk * F_IN], fp32, kind="ExternalInput")
    ein["cvec"] = nc.dram_tensor("cvec", [P, nblk], fp32, kind="ExternalInput")
    ein["bvec"] = nc.dram_tensor("bvec", [P, nblk], fp32, kind="ExternalInput")
    assert n_e % P == 0
    ein["ew"] = nc.dram_tensor("ew", [P, n_e // P], fp32, kind="ExternalInput")
    ein["Wc"] = nc.dram_tensor("Wc", [F_IN, EMB], fp32, kind="ExternalInput")
    ein["Wv"] = nc.dram_tensor("Wv", [F_IN, EMB], fp32, kind="ExternalInput")
    ein["W1o1"] = nc.dram_tensor("W1o1", [EMB, EMB], fp32, kind="ExternalInput")
    ein["W1o2"] = nc.dram_tensor("W1o2", [EMB, EMB], fp32, kind="ExternalInput")
    ein["W2o1"] = nc.dram_tensor("W2o1", [EMB, 1], fp32, kind="ExternalInput")
    ein["W2o2"] = nc.dram_tensor("W2o2", [EMB, 1], fp32, kind="ExternalInput")
    o1 = nc.dram_tensor("o1", [P, nblk], fp32, kind="ExternalOutput")
    o2 = nc.dram_tensor("o2", [P, nblk], fp32, kind="ExternalOutput")

    # ---- internal DRAM
    cons_tab = nc.dram_tensor("cons_tab", [tabr * ncores, EMB], fp32,
                              kind="Internal", addr_space="Shared")
    var2_tab = nc.dram_tensor("var2_tab", [tabr * ncores, EMB], fp32,
                              kind="Internal", addr_space="Shared")
    bounce_c = nc.dram_tensor("bounce_c", [tabr, EMB], fp32, kind="Internal")
    bounce_v = nc.dram_tensor("bounce_v", [tabr, EMB], fp32, kind="Internal")

    iota_d = nc.inline_tensor(
        np.tile(np.arange(P, dtype=np.float32), (P, 1)), "iota_d")
    ident_d = nc.inline_tensor(np.eye(P, dtype=np.float32), "ident_d")
    ones_r_d = nc.inline_tensor(np.ones((1, P), np.float32), "ones_r_d")
    ones_c_d = nc.inline_tensor(np.ones((P, 1), np.float32), "ones_c_d")

    # ---- persistent SBUF
    sb = {}
    def sball(name, shape, dt=fp32):
        sb[name] = nc.alloc_sbuf_tensor("s_" + name, list(shape), dt)
        return sb[name]

    sball("idx0", [P, l_h[0] // 16], i16)
    sball("idx1", [P, l_h[1] // 16], i16)
    sball("w0", [P, c_h[0]])
    sball("w1", [P, c_h[1]])
    sball("sf0", [P, c_h[0]])
    sball("sf1", [P, c_h[1]])
    sball("varA", [P, nblk * EMB])
    sball("varB", [P, nblk * EMB])
    sball("consA", [P, nblk * EMB])
    sball("consB", [P, nblk * EMB])
    sball("dtmp", [P, nblk * EMB])
    sball("etmp", [P, nblk * EMB])
    sball("c_sb", [P, nblk])
    sball("b_sb", [P, nblk])
    sball("o1_sb", [P, nblk])
    sball("o2_sb", [P, nblk])
    sball("iota", [P, P])
    sball("ident", [P, P])
    sball("ones_r", [1, P])
    sball("ones_c", [P, 1])
    sball("inv128", [P, 1])
    sball("zero64", [P, EMB])
    sball("Wc", [F_IN, EMB])
    sball("Wv", [F_IN, EMB])
    sball("W1o1", [EMB, EMB])
    sball("W1o2", [EMB, EMB])
    sball("W2o1", [EMB, 1])
    sball("W2o2", [EMB, 1])

    AG = dict(kind="AllGather", op=mybir.AluOpType.bypass,
              replica_groups=[list(range(ncores))])

    with tile.TileContext(nc) as tc:
        import contextlib
        ctx = contextlib.ExitStack()
        with ctx:
            mpool = ctx.enter_context(
                tc.tile_pool(name="mpsum", bufs=3, space="PSUM"))
            spool = ctx.enter_context(
                tc.tile_pool(name="spsum", bufs=4, space="PSUM"))
            gpool = ctx.enter_context(tc.tile_pool(name="gpool", bufs=4))
            gwpool = ctx.enter_context(tc.tile_pool(name="gwpool", bufs=4))
            selpool = ctx.enter_context(tc.tile_pool(name="selpool", bufs=4))
            smallp = ctx.enter_context(tc.tile_pool(name="smallp", bufs=4))

            dma = nc.sync.dma_start

            # ---- load constants / inputs
            for k in ("idx0", "idx1", "w0", "w1", "sf0", "sf1"):
                dma(sb[k][:], ein[k][:])
            dma(sb["c_sb"][:], ein["cvec"][:])
            dma(sb["b_sb"][:], ein["bvec"][:])
            dma(sb["iota"][:], iota_d[:])
            dma(sb["ident"][:], ident_d[:])
            dma(sb["ones_r"][:], ones_r_d[:])
            dma(sb["ones_c"][:], ones_c_d[:])
            nc.vector.memset(sb["zero64"][:], 0.0)
            for k in ("Wc", "Wv", "W1o1", "W1o2", "W2o1", "W2o2"):
                dma(sb[k][:], ein[k][:])

            # ---- edge-weight norm: inv128 = 1/||w|| broadcast to [128,1]
            with tc.tile_pool(name="ipool", bufs=1) as ipool:
                ew_sb = ipool.tile([P, n_e // P], fp32)
                dma(ew_sb[:], ein["ew"][:])
                sq = ipool.tile([P, n_e // P], fp32)
                ssum = ipool.tile([P, 1], fp32)
                nc.scalar.activation(
                    sq[:], ew_sb[:], mybir.ActivationFunctionType.Square,
                    accum_out=ssum[:])
                ps1 = mpool.tile([1, 1], fp32, tag="m")
                nc.tensor.matmul(ps1[:], lhsT=ssum[:], rhs=sb["ones_c"][:],
                                 start=True, stop=True)
                rinv = ipool.tile([1, 1], fp32)
                nc.vector.reciprocal(rinv[:], ps1[:])
                rsq = ipool.tile([1, 1], fp32)
                nc.scalar.activation(rsq[:], rinv[:],
                                     mybir.ActivationFunctionType.Sqrt)
                ps2 = mpool.tile([P, 1], fp32, tag="m")
                nc.tensor.matmul(ps2[:], lhsT=sb["ones_r"][:], rhs=rsq[:],
                                 start=True, stop=True)
                nc.vector.tensor_copy(sb["inv128"][:], ps2[:])
                for k, ch in (("w0", c_h[0]), ("w1", c_h[1])):
                    nc.vector.tensor_tensor(
                        out=sb[k][:], in0=sb[k][:],
                        in1=sb["inv128"][:].to_broadcast([P, ch]),
                        op=mybir.AluOpType.mult)

                # ---- init: cons/var = relu(feat @ W + b)  (biases are zero)
                for feat_in, wkey, dest in (
                    (ein["cf"], "Wc", "consA"), (ein["vf"], "Wv", "varA")):
                    fsb = ipool.tile([P, nblk * F_IN], fp32)
                    dma(fsb[:], feat_in[:])
                    fview = fsb[:].rearrange("p (t k) -> p t k", k=F_IN)
                    for t in range(nblk):
                        tp = mpool.tile([F_IN, P], fp32, tag="m")
                        nc.tensor.transpose(tp[:], fview[:, t, :], sb["ident"][:])
                        xt = smallp.tile([F_IN, P], fp32)
                        nc.vector.tensor_copy(xt[:], tp[:])
                        mm = mpool.tile([P, EMB], fp32, tag="m")
                        nc.tensor.matmul(mm[:], lhsT=xt[:], rhs=sb[wkey][:],
                                         start=True, stop=True)
                        nc.scalar.activation(
                            sb[dest][:, t * EMB:(t + 1) * EMB], mm[:],
                            mybir.ActivationFunctionType.Relu)

            # initial cons AllGather
            dma(bounce_c[:].rearrange("(p t) f -> p (t f)", p=P),
                sb["consA"][:])
            nc.gpsimd.collective_compute(
                ins=[bounce_c[:]], outs=[cons_tab[:]], **AG)

            # ---- spmm helper
            tab_half = {
                "cons": (cons_tab[0:half_rows, :], cons_tab[half_rows:, :]),
                "var2": (var2_tab[0:half_rows, :], var2_tab[half_rows:, :]),
            }
            idx_sb = (sb["idx0"], sb["idx1"])
            w_sb = (sb["w0"], sb["w1"])
            sf_sb = (sb["sf0"], sb["sf1"])

            def run_spmm(tab_key, svec, s2):
                """conv over this core's blocks; dtmp[:,b] = (conv-svec_b)*s2."""
                import os as _os
                _mode = _os.environ.get("KDEBUG_SPMM", "full")
                if _mode == "skip":
                    nc.vector.memset(sb["dtmp"][:], 0.0)
                    return
                gw_cache = {}
                sel_cache = {}

                def get_gw(h, g):
                    if (h, g) in gw_cache:
                        return gw_cache[(h, g)]
                    nchk = min(BATCH, c_h[h] - g * BATCH)
                    ni = nchk * P
                    gt = gpool.tile([P, BATCH, EMB], fp32, tag="g")
                    nc.gpsimd.dma_gather(
                        out_ap=gt[:, 0:nchk, :],
                        in_ap=tab_half[tab_key][h],
                        idxs_ap=idx_sb[h][:, g * BATCH * 8:
                                          g * BATCH * 8 + ni // 16],
                        num_idxs=ni, num_idxs_reg=ni, elem_size=EMB)
                    gwt = gwpool.tile([P, BATCH, EMB], bf16, tag="gw")
                    nc.vector.tensor_tensor(
                        out=gwt[:, 0:nchk, :], in0=gt[:, 0:nchk, :],
                        in1=w_sb[h][:, g * BATCH:g * BATCH + nchk]
                        .to_broadcast([P, nchk, EMB]),
                        op=mybir.AluOpType.mult)
                    gw_cache[(h, g)] = gwt
                    return gwt

                def get_sel(h, q):
                    if (h, q) in sel_cache:
                        return sel_cache[(h, q)]
                    ng = min(4, c_h[h] - q * 4)
                    st = selpool.tile([P, 4, P], bf16, tag="sel")
                    nc.vector.tensor_tensor(
                        out=st[:, 0:ng, :],
                        in0=sf_sb[h][:, q * 4:q * 4 + ng]
                        .to_broadcast([P, ng, P]),
                        in1=sb["iota"][:].rearrange("p (a f) -> p a f", a=1)
                        .to_broadcast([P, ng, P]),
                        op=mybir.AluOpType.is_equal)
                    sel_cache[(h, q)] = st
                    return st

                if _mode == "gather":
                    for h in range(2):
                        for g in range((c_h[h] + BATCH - 1) // BATCH):
                            get_gw(h, g)
                    nc.vector.memset(sb["dtmp"][:], 0.0)
                    return
                for b in range(nblk):
                    chunks = block_chunks[b]
                    if not chunks:
                        nc.vector.tensor_scalar(
                            out=sb["dtmp"][:, b * EMB:(b + 1) * EMB],
                            in0=sb["zero64"][:],
                            scalar1=svec[:, b:b + 1], scalar2=s2,
                            op0=mybir.AluOpType.subtract,
                            op1=mybir.AluOpType.mult)
                        continue
                    ps = spool.tile([P, EMB], fp32, tag="acc")
                    for i, (h, ci) in enumerate(chunks):
                        gwt = get_gw(h, ci // BATCH)
                        st = get_sel(h, ci // 4)
                        nc.tensor.matmul(
                            ps[:], lhsT=st[:, ci % 4, :],
                            rhs=gwt[:, ci % BATCH, :],
                            start=(i == 0), stop=(i == len(chunks) - 1))
                    nc.vector.tensor_scalar(
                        out=sb["dtmp"][:, b * EMB:(b + 1) * EMB], in0=ps[:],
                        scalar1=svec[:, b:b + 1], scalar2=s2,
                        op0=mybir.AluOpType.subtract,
                        op1=mybir.AluOpType.mult)

            relu = mybir.ActivationFunctionType.Relu
            var_cur, var_nxt = sb["varA"], sb["varB"]
            cons_cur, cons_nxt = sb["consA"], sb["consB"]

            import os
            _phase = os.environ.get("KDEBUG_PHASE", "full")
            _nlayers = {"init": 0, "spmm1": 1, "layer1": 1, "full": N_LAYERS}.get(_phase, N_LAYERS)
            _spmm1_only = _phase == "spmm1"
            _do_heads = _phase == "full"

            for layer in range(_nlayers):
                # conv = spmm(cons); dtmp = t1*(c - conv) ... via (conv-c)*(-t1)
                run_spmm("cons", sb["c_sb"], -t_ctov[layer])
                if _spmm1_only:
                    nc.vector.tensor_copy(sb["o1_sb"][:], sb["dtmp"][:, 0:nblk])
                    nc.vector.tensor_copy(sb["o2_sb"][:], sb["dtmp"][:, 0:nblk])
                    break
                nc.vector.tensor_tensor(out=sb["etmp"][:], in0=var_cur[:],
                                        in1=sb["dtmp"][:],
                                        op=mybir.AluOpType.add)
                nc.scalar.activation(var_nxt[:], sb["etmp"][:], relu,
                                     scale=float(SCALE))
                nc.scalar.activation(sb["dtmp"][:], sb["etmp"][:], relu,
                                     scale=float(2.0 * SCALE))
                nc.vector.tensor_tensor(out=sb["etmp"][:], in0=sb["dtmp"][:],
                                        in1=var_cur[:],
                                        op=mybir.AluOpType.subtract)
                dma(bounce_v[:].rearrange("(p t) f -> p (t f)", p=P),
                    sb["etmp"][:])
                nc.gpsimd.collective_compute(
                    ins=[bounce_v[:]], outs=[var2_tab[:]], **AG)

                # conv2 = spmm(var2); dtmp = t0*(conv2 - b)
                run_spmm("var2", sb["b_sb"], t_vtoc[layer])
                nc.vector.tensor_tensor(out=sb["etmp"][:], in0=cons_cur[:],
                                        in1=sb["dtmp"][:],
                                        op=mybir.AluOpType.add)
                nc.scalar.activation(cons_nxt[:], sb["etmp"][:], relu)
                if layer < N_LAYERS - 1:
                    dma(bounce_c[:].rearrange("(p t) f -> p (t f)", p=P),
                        cons_nxt[:])
                    nc.gpsimd.collective_compute(
                        ins=[bounce_c[:]], outs=[cons_tab[:]], **AG)
                var_cur, var_nxt = var_nxt, var_cur
                cons_cur, cons_nxt = cons_nxt, cons_cur

            # ---- output heads
            if not _do_heads and not _spmm1_only:
                nc.vector.tensor_copy(sb["o1_sb"][:], var_cur[:, 0:nblk])
                nc.vector.tensor_copy(sb["o2_sb"][:], cons_cur[:, 0:nblk])
            for featbuf, w1k, w2k, outsb in (() if not _do_heads else (
                (var_cur, "W1o1", "W2o1", "o1_sb"),
                (cons_cur, "W1o2", "W2o2", "o2_sb"))):
                fview = featbuf[:].rearrange("p (t f) -> p t f", f=EMB)
                for t in range(nblk):
                    tp = mpool.tile([EMB, P], fp32, tag="m")
                    nc.tensor.transpose(tp[:], fview[:, t, :], sb["ident"][:])
                    vt = smallp.tile([EMB, P], fp32, tag="vt")
                    nc.vector.tensor_copy(vt[:], tp[:])
                    hm = mpool.tile([EMB, P], fp32, tag="m")
                    nc.tensor.matmul(hm[:], lhsT=sb[w1k][:], rhs=vt[:],
                                     start=True, stop=True)
                    hs = smallp.tile([EMB, P], fp32, tag="hs")
                    nc.scalar.activation(hs[:], hm[:], relu)
                    om = mpool.tile([P, 1], fp32, tag="m")
                    nc.tensor.matmul(om[:], lhsT=hs[:], rhs=sb[w2k][:],
                                     start=True, stop=True)
                    nc.vector.tensor_copy(sb[outsb][:, t:t + 1], om[:])

            dma(o1[:], sb["o1_sb"][:])
            dma(o2[:], sb["o2_sb"][:])

    nc.compile()
    return nc


# ---------------------------------------------------------------- entry


def _run_kernel(n, ncores, con_feat, var_feat, edge_weight, c, b,
                W_cons, b_cons, W_var, b_var, temps_ctov, temps_vtoc,
                W1_o1, b1_o1, W2_o1, W1_o2, b1_o2, W2_o2, edge_index):
    global LAST_RESULT
    from concourse.bass_utils import run_bass_kernel_spmd

    for bias in (b_cons, b_var, b1_o1, b1_o2):
        assert not np.any(np.asarray(bias)), "nonzero bias unsupported"

    meta = _prep_edges(edge_index, edge_weight, n, ncores)
    s, nblk = meta["s"], meta["nblk"]
    n_e = int(np.asarray(edge_weight).shape[0])
    scalars = {
        "n_edges": n_e,
        "t_ctov": [float(np.asarray(temps_ctov)[i, 1]) for i in range(N_LAYERS)],
        "t_vtoc": [float(np.asarray(temps_vtoc)[i, 0]) for i in range(N_LAYERS)],
    }
    nc = _build(meta, scalars, ncores)

    ew_l = np.ascontiguousarray(
        np.asarray(edge_weight, np.float32).reshape(P, n_e // P))
    shared = {
        "ew": ew_l,
        "Wc": np.asarray(W_cons, np.float32),
        "Wv": np.asarray(W_var, np.float32),
        "W1o1": np.asarray(W1_o1, np.float32),
        "W1o2": np.asarray(W1_o2, np.float32),
        "W2o1": np.asarray(W2_o1, np.float32),
        "W2o2": np.asarray(W2_o2, np.float32),
    }
    in_maps = []
    for core in range(ncores):
        idx_s, w_s, sf_s = meta["streams"][core]
        sl = slice(core, None, ncores)
        m = {
            "idx0": _idx_layout(idx_s[0]),
            "idx1": _idx_layout(idx_s[1]),
            "w0": _chunk_layout(w_s[0]),
            "w1": _chunk_layout(w_s[1]),
            "sf0": _chunk_layout(sf_s[0]),
            "sf1": _chunk_layout(sf_s[1]),
            "cf": _tile_rows(np.asarray(con_feat[sl], np.float32), s, nblk, F_IN),
            "vf": _tile_rows(np.asarray(var_feat[sl], np.float32), s, nblk, F_IN),
            "cvec": _tile_rows(np.asarray(c[sl], np.float32), s, nblk, 1),
            "bvec": _tile_rows(np.asarray(b[sl], np.float32), s, nblk, 1),
        }
        m.update(shared)
        in_maps.append(m)

    if TRACE:
        try:
            import axon_profile_shim  # noqa: F401
        except ImportError:
            pass
    res = run_bass_kernel_spmd(nc, in_maps, core_ids=list(range(ncores)),
                               trace=TRACE)
    LAST_RESULT = res

    rows = np.arange(s)
    p, t = rows % P, rows // P
    out1 = np.empty((n, 1), np.float32)
    out2 = np.empty((n, 1), np.float32)
    for core in range(ncores):
        out1[core::ncores, 0] = res.results[core]["o1"][p, t]
        out2[core::ncores, 0] = res.results[core]["o2"][p, t]
    return out1, out2


def kernel(con_feat, var_feat, edge_weight, c, b,
           W_cons, b_cons, W_var, b_var, temps_ctov, temps_vtoc,
           W1_o1, b1_o1, W2_o1, W1_o2, b1_o2, W2_o2, edge_index):
    return _run_kernel(
        50000, 8, con_feat, var_feat, edge_weight, c, b,
        W_cons, b_cons, W_var, b_var, temps_ctov, temps_vtoc,
        W1_o1, b1_o1, W2_o1, W1_o2, b1_o2, W2_o2, edge_index)
